# revision 24
# baseline (speedup 1.0000x reference)
"""Trainium2 Bass kernel for nn_CifarBaselineSNN.

conv1(3->64,3x3,p1) -> BN -> LIF -> avgpool2
conv2(64->128,3x3,p1) -> BN -> LIF -> avgpool2
fc1(8192->256) -> LIF -> fc2(256->10)+b
T=8, B=128. Data-parallel over B across 8 NeuronCores (16 samples/core);
BN statistics are global over the batch -> 2 small AllReduces.

v2 layout: conv1 output is never stored to DRAM. Pass A1 computes conv1 only
to accumulate BN1 statistics; after the AllReduce, conv1 is recomputed fused
with the LIF1 scan (DVE reads PSUM directly) and conv2 runs in the same
per-timestep loop, overlapping TensorE and DVE. Weight loads use native-layout
DMAs plus on-chip TensorE transposes to avoid small-descriptor storms.
"""

import sys
import os

for _p in ("/opt/trn_rl_repo", "/root/.axon_site/_ro/trn_rl_repo"):
    if os.path.isdir(_p) and _p not in sys.path:
        sys.path.append(_p)

import numpy as np

import concourse.bass as bass
import concourse.mybir as mybir
import concourse.tile as tile
from concourse import bacc
from concourse import bass_utils
from concourse import masks
from concourse import dve_ops as _dops
from concourse.dve_uop import DveOpSpec
from concourse.dve_spec import (
    Spec, Src0, Src1, C0, C1, C2, Zero, AluOp, sq, select, lower, _has_src1,
)

F32 = mybir.dt.float32
BF16 = mybir.dt.bfloat16
AF = mybir.ActivationFunctionType
ALU = mybir.AluOpType

T = 8
B_FULL = 128
N_CORES = 8
BL = B_FULL // N_CORES  # 16 samples per core
EPS = 1e-5


# --------------------------------------------------------------------------
# Custom DVE ops (fused LIF pieces)
# --------------------------------------------------------------------------

def _register_op(name, spec, ref):
    for op in _dops.OPS:
        if op.name == name:
            return op
    spec = Spec(body=spec.body, accum=spec.accum, accum_init=spec.accum_init,
                reference=ref)
    shas = {}
    for ver in ("v3", "v4"):
        s = DveOpSpec(name=name, opcode=0, uops=lower(spec, ver=ver),
                      rd1_en=_has_src1(spec))
        shas[ver] = s.sha(ver)
    op = _dops.DveOp(name, spec, subdim=False, uops_sha=shas)
    _dops.OPS.append(op)
    _dops.CUSTOM_DVE_SPECS[name] = spec
    _dops._SUB_OPCODE_FOR_NAME[name] = max(_dops._SUB_OPCODE_FOR_NAME.values()) + 1
    return op


# A_t = (A_{t-1} if A_{t-1} < theta_{t-1} else 0) + y*scale + bias
LIF_CHARGE = _register_op(
    "LIF_CHARGE_SNN",
    Spec(body=select(Src0 >= C2, Zero, Src0) + Src1 * C0 + C1),
    lambda in0, in1, s0, s1, imm2: np.where(in0 >= imm2, 0.0, in0) + in1 * s0 + s1,
)

# spike counts over horizontal pairs: (a>=th) + (b>=th)   (values 0/1/2)
SPIKE_HPOOL = _register_op(
    "SPIKE_HPOOL_SNN",
    Spec(body=(Src0 >= C0) + (Src1 >= C0)),
    lambda in0, in1, s0, s1, imm2: (in0 >= s0).astype(np.float32)
    + (in1 >= s0).astype(np.float32),
)

# pooled = (hpA + hpB) * 0.25
VPOOL_SCALE = _register_op(
    "VPOOL_SCALE_SNN",
    Spec(body=(Src0 + Src1) * C1),
    lambda in0, in1, s0, s1, imm2: (in0 + in1) * s1,
)

# pre-scaled spike pair count: ((a>=th)+(b>=th)) * C1  -- lets the final
# 2x2 reduction be a plain ADD on the (otherwise idle) gpsimd engine
HPOOL_SC = _register_op(
    "HPOOL_SC_SNN",
    Spec(body=((Src0 >= C0) + (Src1 >= C0)) * C1),
    lambda in0, in1, s0, s1, imm2: ((in0 >= s0).astype(np.float32)
    + (in1 >= s0).astype(np.float32)) * s1,
)

# square + row-sum (for BN sum-of-squares on the vector engine)
SQ_ACC = _register_op(
    "SQ_ACC_SNN",
    Spec(body=sq(Src0), accum=AluOp.ADD),
    lambda in0, s0, s1, imm2: in0 * in0,
)

# plain spike: (a >= th)
SPIKE_GE = _register_op(
    "SPIKE_GE_SNN",
    Spec(body=(Src0 >= C0) + Zero),
    lambda in0, s0, s1, imm2: (in0 >= s0).astype(np.float32),
)


# --------------------------------------------------------------------------
# Kernel build
# --------------------------------------------------------------------------

def _stats_to_scale_bias(nc, pool, tot, g_dr, b_dr, n_count, nch, out_scale, out_bias):
    """tot: [nch,2] SBUF (sum, sumsq). Writes out_scale/out_bias [nch,8]:
    scale[:,t] = gamma*rstd*2^t ; bias[:,t] = (beta - mu*gamma*rstd)*2^t."""
    mu = pool.tile([nch, 1], F32)
    nc.vector.tensor_scalar_mul(mu[:], tot[:, 0:1], 1.0 / n_count)
    e2 = pool.tile([nch, 1], F32)
    nc.vector.tensor_scalar_mul(e2[:], tot[:, 1:2], 1.0 / n_count)
    var = pool.tile([nch, 1], F32)
    nc.vector.tensor_tensor(var[:], mu[:], mu[:], ALU.mult)
    nc.vector.tensor_tensor(var[:], e2[:], var[:], ALU.subtract)
    nc.vector.tensor_scalar_add(var[:], var[:], float(EPS))
    std = pool.tile([nch, 1], F32)
    nc.scalar.activation(std[:], var[:], AF.Sqrt, bias=0.0, scale=1.0)
    rstd = pool.tile([nch, 1], F32)
    nc.vector.reciprocal(rstd[:], std[:])
    gam = pool.tile([nch, 1], F32)
    nc.sync.dma_start(gam[:], g_dr.ap()[:, None])
    bet = pool.tile([nch, 1], F32)
    nc.sync.dma_start(bet[:], b_dr.ap()[:, None])
    gr = pool.tile([nch, 1], F32)
    nc.vector.tensor_tensor(gr[:], gam[:], rstd[:], ALU.mult)
    bb = pool.tile([nch, 1], F32)  # beta - mu*gr
    nc.vector.tensor_tensor(bb[:], mu[:], gr[:], ALU.mult)
    nc.vector.tensor_tensor(bb[:], bet[:], bb[:], ALU.subtract)
    for t in range(T):
        nc.vector.tensor_scalar_mul(out_scale[:nch, t : t + 1], gr[:], float(2.0**t))
        nc.vector.tensor_scalar_mul(out_bias[:nch, t : t + 1], bb[:], float(2.0**t))


def _allreduce(nc, dram_pool, sb_pool, src_ap, shape):
    """AllReduce-add src_ap ([P,F] SBUF) across all 8 cores; returns SBUF tile."""
    bin_ = dram_pool.tile(list(shape), F32)
    bout = dram_pool.tile(list(shape), F32)
    nc.gpsimd.dma_start(bin_[:], src_ap)
    nc.gpsimd.collective_compute(
        "AllReduce", ALU.add,
        replica_groups=[list(range(N_CORES))],
        ins=[bin_.opt()], outs=[bout.opt()],
    )
    res = sb_pool.tile(list(shape), F32)
    nc.gpsimd.dma_start(res[:], bout[:])
    return res


def build(nc):
    # ---- DRAM I/O -------------------------------------------------------
    x_seq = nc.dram_tensor("x_seq", [T, BL, 3, 32, 32], F32, kind="ExternalInput")
    w1_dr = nc.dram_tensor("conv1_w", [64, 3, 3, 3], F32, kind="ExternalInput")
    g1_dr = nc.dram_tensor("bn1_g", [64], F32, kind="ExternalInput")
    b1_dr = nc.dram_tensor("bn1_b", [64], F32, kind="ExternalInput")
    w2_dr = nc.dram_tensor("conv2_w", [128, 64, 3, 3], F32, kind="ExternalInput")
    g2_dr = nc.dram_tensor("bn2_g", [128], F32, kind="ExternalInput")
    b2_dr = nc.dram_tensor("bn2_b", [128], F32, kind="ExternalInput")
    fc1_dr = nc.dram_tensor("fc1_w", [256, 8192], F32, kind="ExternalInput")
    fc2_dr = nc.dram_tensor("fc2_w", [10, 256], F32, kind="ExternalInput")
    fc2b_dr = nc.dram_tensor("fc2_b", [10], F32, kind="ExternalInput")
    out_dr = nc.dram_tensor("out", [T, BL, 10], F32, kind="ExternalOutput")

    with tile.TileContext(nc) as tc:
        import contextlib
        with contextlib.ExitStack() as ctx:
            dram = ctx.enter_context(tc.tile_pool(name="dram", bufs=1, space="DRAM"))
            persist = ctx.enter_context(tc.tile_pool(name="persist", bufs=1))

            # DRAM scratch: conv2 pre-BN output, [t][ch][sample][pos]
            y2_dram = dram.tile([T, 128, BL, 256], BF16)

            # persistent small tensors
            scale1 = persist.tile([128, T], F32)
            bias1 = persist.tile([128, T], F32)
            scale2 = persist.tile([128, T], F32)
            bias2 = persist.tile([128, T], F32)
            s1buf = persist.tile([128, 32], F32)
            s2buf = persist.tile([128, 32], F32)
            s1buf2 = persist.tile([128, 32], F32)
            s2buf2 = persist.tile([128, 32], F32)
            ident = persist.tile([128, 128], F32)
            masks.make_identity(nc, ident[:])
            ident_bf = persist.tile([128, 128], BF16)
            masks.make_identity(nc, ident_bf[:])

            # conv1 weights: row r = g*3+ci (g=dy*3+dx); bf16 (conv1 runs in
            # pure bf16 -- layer-1 activations tolerate ~0.5% rounding, far
            # inside the LIF threshold margins)
            w1f = persist.tile([27, 64], F32)
            for dy in range(3):
                for dx in range(3):
                    r0 = dy * 9 + dx * 3
                    nc.sync.dma_start(
                        w1f[r0 : r0 + 3, :],
                        w1_dr.ap()[:, :, dy, dx].rearrange("c ci -> ci c"),
                    )
            w1b = persist.tile([27, 64], BF16)
            nc.vector.tensor_copy(w1b[:], w1f[:])

            # conv2 weights: native load [c,(ci g)] then 9 TensorE transposes
            # of the stride-9 column slices -> psum [64ci, 128c]; hi/lo split.
            # w2cat rows 0-63 = W_hi(ci), 64-127 = W_lo(ci).
            w2cat = ctx.enter_context(tc.tile_pool(name="w2pool", bufs=1)).tile(
                [128, 9, 128], BF16)
            with tc.tile_pool(name="w2stage", bufs=1) as w2st, \
                 tc.tile_pool(name="w2ps", bufs=2, space="PSUM") as w2ps:
                w2nat = w2st.tile([128, 576], F32)
                nc.sync.dma_start(w2nat[:], w2_dr.ap().rearrange("c ci dy dx -> c (ci dy dx)"))
                w2f2 = w2st.tile([64, 9, 128], F32)
                for g in range(9):
                    psT = w2ps.tile([128, 512], F32, tag="w2t", name="w2t")
                    nc.tensor.transpose(psT[0:64, 0:128],
                                        w2nat[:, g : 576 : 9], ident[:, 0:128])
                    nc.scalar.copy(w2f2[:, g, :], psT[0:64, 0:128])
                w2lo = w2st.tile([64, 9, 128], BF16)
                nc.vector.tensor_copy(w2cat[0:64], w2f2[:])
                nc.vector.tensor_tensor(w2lo[:], w2f2[:], w2cat[0:64], ALU.subtract)
                nc.sync.dma_start(w2cat[64:128], w2lo[:])

            # =============== x staging: padded bf16 hi/lo planes in DRAM ====
            GUARD = 64
            TPLANE = BL * 1156  # one timestep's stream of padded 34x34 frames
            TSTR = GUARD + TPLANE + GUARD
            # per-t DRAM tiles: imc(t) only depends on its own 3 staging DMAs
            xbase = [dram.tile([3, TSTR], BF16, name=f"xb{t}") for t in range(T)]
            st_engs = [nc.sync, nc.scalar, nc.gpsimd]
            with tc.tile_pool(name="xstage", bufs=1) as xst:
                xpadF = xst.tile([128, 3, 34, 34], F32)
                nc.vector.memset(xpadF[:], 0.0)
                xraw = xst.tile([128, 3, 32, 32], F32)
                nc.sync.dma_start(
                    xraw.rearrange("p c h w -> p (c h w)"),
                    x_seq.ap().rearrange("t b c h w -> (t b) (c h w)"),
                )
                nc.scalar.copy(xpadF[:, :, 1:33, 1:33], xraw[:])
                xpad_flat = xpadF.rearrange("p c h w -> p (c h w)")
                x_hiF = xst.tile([128, 3, 34, 34], BF16)
                xhi_flat = x_hiF.rearrange("p c h w -> p (c h w)")
                nc.vector.tensor_copy(xhi_flat[:], xpad_flat[:])
                for t in range(T):
                    for ci in range(3):
                        st_engs[(3 * t + ci) % 3].dma_start(
                            xbase[t][ci, GUARD : GUARD + TPLANE]
                                .rearrange("(b f) -> b f", f=1156),
                            x_hiF[t * BL : (t + 1) * BL, ci]
                                .rearrange("p h w -> p (h w)"),
                        )

            # im2col row r = g*3+ci reads plane ci at shift (dy-1)*34+(dx-1);
            # one 2-d AP [ci:TSTR x cols:1] per 3-row block. Per-t tiles.
            TCOLS = TPLANE

            def imc_src(t, dy, dx):
                a = xbase[t][:]
                off = (a.offset + GUARD + (dy - 1) * 34 + (dx - 1))
                return bass.AP(a.tensor, off, [[TSTR, 3], [1, TCOLS]])

            imc_ctx = contextlib.ExitStack()
            imc_pool = imc_ctx.enter_context(tc.tile_pool(name="imc", bufs=2))

            imc_engs = [nc.sync, nc.scalar, nc.gpsimd]

            def load_imc(t):
                imc = imc_pool.tile([27, TCOLS], BF16, tag="imc", name="imc")
                for g in range(9):
                    dy, dx = g // 3, g % 3
                    r0 = 3 * g
                    imc_engs[g % 3].dma_start(imc[r0 : r0 + 3, :],
                                              imc_src(t, dy, dx))
                return imc.rearrange("p (b h w) -> p b h w", h=34, w=34)

            def conv1_mm(ps, imc_v, p):
                # ps [128,1024] = [2 samples x 64ch, 32x32]; 4 matmuls
                for bhalf in range(2):
                    fr = 2 * p + bhalf
                    for hh in range(2):
                        h0 = hh * 16
                        rhs = imc_v[:, fr, h0 + 1 : h0 + 17, 1:33]
                        nc.tensor.matmul(
                            ps[64 * bhalf : 64 * bhalf + 64,
                               512 * hh : 512 * hh + 512],
                            w1b[:, :], rhs[0:27],
                            start=True, stop=True,
                            tile_position=(0, 64 * bhalf),
                        )

            # =============== PASS A1: conv1 -> BN1 stats only ===============
            # 4 samples per [128,2048] psum tile halves the per-drain
            # semaphore chains (stats ops amortize handoff latency better)
            with tc.tile_pool(name="psumA1", bufs=2, space="PSUM") as psA1, \
                 tc.tile_pool(name="yscr", bufs=2) as yscr_pool, \
                 tc.tile_pool(name="sqscr", bufs=2) as sq_pool:
                for t in range(T):
                    imc_v = load_imc(t)
                    for pp in range(4):
                        idx = t * 4 + pp
                        ps = psA1.tile([128, 2048], F32, tag="ps", name="psA1")
                        for jj in range(4):
                            fr = 4 * pp + jj
                            for hh in range(2):
                                h0 = hh * 16
                                rhs = imc_v[:, fr, h0 + 1 : h0 + 17, 1:33]
                                nc.tensor.matmul(
                                    ps[64 * (jj % 2) : 64 * (jj % 2) + 64,
                                       (jj // 2) * 1024 + 512 * hh :
                                       (jj // 2) * 1024 + 512 * hh + 512],
                                    w1b[:, :], rhs[0:27],
                                    start=True, stop=True,
                                    tile_position=(0, 64 * (jj % 2)),
                                )
                        y_scr = yscr_pool.tile([128, 2048], F32, name="yscr")
                        nc.scalar.activation(y_scr[:], ps[:], AF.Identity,
                                             bias=0.0, scale=1.0,
                                             accum_out=s1buf[:, idx : idx + 1])
                        sq_t = sq_pool.tile([128, 2048], F32, name="sqscr")
                        nc.vector._custom_dve(
                            SQ_ACC, out=sq_t[:], in0=ps[:],
                            accum_out=s2buf[:, idx : idx + 1])

            # =============== BN1 stats + allreduce ===============
            sums1 = persist.tile([128, 2], F32)
            nc.vector.tensor_reduce(sums1[:, 0:1], s1buf[:], mybir.AxisListType.X, ALU.add)
            nc.vector.tensor_reduce(sums1[:, 1:2], s2buf[:], mybir.AxisListType.X, ALU.add)
            g1 = _allreduce(nc, dram, persist, sums1[:], (128, 2))
            par1 = persist.tile([64, 2], F32)
            nc.sync.dma_start(par1[:], g1[64:128, :])
            tot1 = persist.tile([64, 2], F32)
            nc.vector.tensor_tensor(tot1[:], g1[0:64, :], par1[:], ALU.add)
            _stats_to_scale_bias(nc, persist, tot1, g1_dr, b1_dr,
                                 float(T * B_FULL * 32 * 32), 64, scale1, bias1)
            nc.sync.dma_start(scale1[64:128, :], scale1[0:64, :])
            nc.sync.dma_start(bias1[64:128, :], bias1[0:64, :])

            # =============== FUSED A2+B+C: conv1 -> LIF1 -> pool -> conv2 ===
            with tc.tile_pool(name="stfused", bufs=1) as stp, \
                 tc.tile_pool(name="p1pool", bufs=1) as p1p, \
                 tc.tile_pool(name="psumA2", bufs=2, space="PSUM") as psA2, \
                 tc.tile_pool(name="psumC", bufs=2, space="PSUM") as psC, \
                 tc.tile_pool(name="hp1", bufs=1) as hp_pool, \
                 tc.tile_pool(name="dup", bufs=2) as dup_pool, \
                 tc.tile_pool(name="ysb2", bufs=1) as ysb2_pool, \
                 tc.tile_pool(name="sq2", bufs=1) as sq2_pool:
                st = [stp.tile([128, 8, 1024], F32, name=f"st{i}") for i in range(2)]
                nc.vector.memset(st[0].rearrange("p a b -> p (a b)"), 0.0)
                pooled1 = [p1p.tile([128, 8, 18, 18], BF16, name=f"p1_{i}")
                           for i in range(2)]
                nc.vector.memset(pooled1[0].rearrange("p a h w -> p (a h w)"), 0.0)
                nc.vector.memset(pooled1[1].rearrange("p a h w -> p (a h w)"), 0.0)

                for t in range(T):
                    imc_v = load_imc(t)
                    a_old, a_new = st[t % 2], st[(t + 1) % 2]
                    p1t = pooled1[t % 2]
                    for p in range(8):
                        ps = psA2.tile([128, 1024], F32, tag="ps2", name="psA2")
                        conv1_mm(ps, imc_v, p)
                        nc.vector._custom_dve(
                            LIF_CHARGE, out=a_new[:, p, :], in0=a_old[:, p, :],
                            in1=ps[:],
                            s0=scale1[:, t : t + 1], s1=bias1[:, t : t + 1],
                            imm2=float(2.0**t),
                        )
                        av = a_new[:, p, :].rearrange("p (h w) -> p h w", h=32)
                        hp = hp_pool.tile([128, 32, 16], F32, tag="hp", name="hp")
                        nc.vector._custom_dve(
                            HPOOL_SC, out=hp[:],
                            in0=av[:, :, 0:32:2], in1=av[:, :, 1:32:2],
                            s0=float(2.0 ** (t + 1)), s1=0.25,
                        )
                        nc.vector._custom_dve(
                            VPOOL_SCALE,
                            out=p1t[:, p, 1:17, 1:17],
                            in0=hp[:, 0:32:2, :], in1=hp[:, 1:32:2, :],
                            s1=1.0,
                        )
                    # conv2 for this timestep: duplicate (ci | ci) partitions,
                    # samples blocked evens (slots 0-7) then odds (slots 8-15)
                    dup = dup_pool.tile([128, 16, 324], BF16, tag="dup", name="dup")
                    p1flat = [p1t[0:64].rearrange("p a h w -> p a (h w)"),
                              p1t[64:128].rearrange("p a h w -> p a (h w)")]
                    nc.sync.dma_start(dup[0:64, 0:8, :], p1flat[0])
                    nc.scalar.dma_start(dup[64:128, 0:8, :], p1flat[0])
                    nc.gpsimd.dma_start(dup[0:64, 8:16, :], p1flat[1])
                    nc.sync.dma_start(dup[64:128, 8:16, :], p1flat[1])
                    dup_v = dup.rearrange("p s (h w) -> p s h w", h=18)
                    y_sb = ysb2_pool.tile([128, BL, 256], BF16, tag="ysb",
                                          name="ysb2")
                    for sp2 in range(4):
                        cidx = t * 4 + sp2
                        ps2 = psC.tile([128, 1024], F32, tag="psc", name="psC")
                        for half2 in range(2):
                            j0 = 2 * sp2 + half2
                            for g in range(9):
                                dy, dx = g // 3, g % 3
                                nc.tensor.matmul(
                                    ps2[:, 512 * half2 : 512 * half2 + 512],
                                    w2cat[:, g, :],
                                    dup_v[:, j0 : j0 + 9 : 8,
                                          dy : dy + 16, dx : dx + 16],
                                    start=(g == 0), stop=(g == 8),
                                )
                        nc.scalar.activation(
                            y_sb[:, 4 * sp2 : 4 * sp2 + 4, :], ps2[:],
                            AF.Identity, bias=0.0, scale=1.0,
                            accum_out=s1buf2[:, cidx : cidx + 1])
                        sq_t = sq2_pool.tile([128, 1024], F32, name="sq2")
                        nc.vector._custom_dve(
                            SQ_ACC, out=sq_t[:], in0=ps2[:],
                            accum_out=s2buf2[:, cidx : cidx + 1])
                    st_engs[t % 3].dma_start(y2_dram[t], y_sb[:])

            imc_ctx.close()  # free the 74KB/partition im2col buffers

            # =============== BN2 stats + allreduce ===============
            sums2 = persist.tile([128, 2], F32)
            nc.vector.tensor_reduce(sums2[:, 0:1], s1buf2[:], mybir.AxisListType.X, ALU.add)
            nc.vector.tensor_reduce(sums2[:, 1:2], s2buf2[:], mybir.AxisListType.X, ALU.add)
            g2 = _allreduce(nc, dram, persist, sums2[:], (128, 2))
            _stats_to_scale_bias(nc, persist, g2, g2_dr, b2_dr,
                                 float(T * B_FULL * 16 * 16), 128, scale2, bias2)

            # fc weights: native loads + TensorE transposes -> fp32 lhsT tiles
            # fc1wT[:, hw, m, :] = fc1_w[m*128:(m+1)*128, c*64+hw]^T  ([c, o])
            fcpool = ctx.enter_context(tc.tile_pool(name="fcpool", bufs=1))
            fc1wT = fcpool.tile([128, 64, 2, 128], BF16)
            fc2w = fcpool.tile([128, 2, 10], F32)
            fc2b = fcpool.tile([10, 1], F32)
            nc.sync.dma_start(fc2b[:], fc2b_dr.ap()[:, None])
            with tc.tile_pool(name="fcstage", bufs=1) as fst, \
                 tc.tile_pool(name="psumT", bufs=4, space="PSUM") as psT_pool:
                fc2nat = fst.tile([10, 256], F32)
                nc.sync.dma_start(fc2nat[:], fc2_dr.ap())
                for m in range(2):
                    psT = psT_pool.tile([128, 512], F32, tag="fct", name="fct")
                    nc.tensor.transpose(psT[:, 0:10],
                                        fc2nat[:, m * 128 : (m + 1) * 128],
                                        ident[0:10, 0:10])
                    nc.scalar.copy(fc2w[:, m, :], psT[:, 0:10])
                for m in range(2):
                    # bf16 casting load (gpsimd-only feature); primed from the
                    # END OF A1 stats so it streams during the fused conv loop
                    fc1nat = fst.tile([128, 8192], BF16, tag="fcn", name="fcn")
                    nc.gpsimd.dma_start(fc1nat[0:1, 0:1], s2buf[0:1, 31:32])
                    nc.gpsimd.dma_start(fc1nat[:], fc1_dr.ap()[m * 128 : (m + 1) * 128, :])
                    for hw in range(64):
                        psT = psT_pool.tile([128, 512], BF16, tag="fctb", name="fctb")
                        nc.tensor.transpose(psT[:, 0:128],
                                            fc1nat[:, hw : 8192 : 64],
                                            ident_bf[:, 0:128])
                        nc.scalar.copy(fc1wT[:, hw, m, :], psT[:, 0:128])

                # ======== STAGE D: LIF2 + pool (all 16 samples per op) ======
                pooled2 = fcpool.tile([128, T, BL, 8, 8], BF16)
                with tc.tile_pool(name="stageD", bufs=2) as pD, \
                     tc.tile_pool(name="stD", bufs=1) as stD, \
                     tc.tile_pool(name="hp2", bufs=1) as hp2_pool:
                    st2 = [stD.tile([128, 4096], F32, name=f"st2_{i}") for i in range(2)]
                    nc.vector.memset(st2[0][:], 0.0)
                    for t in range(T):
                        yc = pD.tile([128, 4096], BF16, tag="yc", name="ycD")
                        st_engs[t % 3].dma_start(
                            yc.rearrange("p (b c) -> p b c", b=BL),
                            y2_dram[t],
                        )
                        a_old, a_new = st2[t % 2], st2[(t + 1) % 2]
                        nc.vector._custom_dve(
                            LIF_CHARGE, out=a_new[:], in0=a_old[:], in1=yc[:],
                            s0=scale2[:, t : t + 1], s1=bias2[:, t : t + 1],
                            imm2=float(2.0**t),
                        )
                        # state flat = (b,h,w); (b h) collapses to one stride-16 dim
                        av = a_new.rearrange("p (bh w) -> p bh w", w=16)
                        hp2 = hp2_pool.tile([128, BL * 16, 8], F32, tag="hp2", name="hp2")
                        nc.vector._custom_dve(
                            HPOOL_SC, out=hp2[:],
                            in0=av[:, :, 0:16:2], in1=av[:, :, 1:16:2],
                            s0=float(2.0 ** (t + 1)), s1=0.25,
                        )
                        # hp2 flat = (b,h,w2): rows h even/odd pair up as
                        # offset j*16+{0..7} and j*16+{8..15} with j=(b,h')
                        hv = hp2.rearrange("p j w -> p (j w)").rearrange(
                            "p (j k) -> p j k", k=16)
                        p2o = pooled2.rearrange("p t b h w -> p (t b h) w")
                        nc.vector._custom_dve(
                            VPOOL_SCALE,
                            out=p2o[:, t * BL * 8 : (t + 1) * BL * 8, :],
                            in0=hv[:, :, 0:8], in1=hv[:, :, 8:16],
                            s1=1.0,
                        )

            # =============== STAGE E: fc1 (fp32) + LIF + fc2 ===============
            p2v = pooled2.rearrange("p t b h w -> p (t b) (h w)")
            with tc.tile_pool(name="stageE", bufs=1) as pE, \
                 tc.tile_pool(name="psumE", bufs=2, space="PSUM") as psE:
                s_sb = pE.tile([128, 2, T, BL], F32)
                for m in range(2):
                    psf = psE.tile([128, 128], F32, tag="psf")
                    for hw in range(64):
                        nc.tensor.matmul(
                            psf[:], fc1wT[:, hw, m, :], p2v[:, :, hw],
                            start=(hw == 0), stop=(hw == 63),
                        )
                    stf = [pE.tile([128, BL], F32, tag=f"stf{i}", name=f"stf{i}")
                           for i in range(2)]
                    nc.vector.memset(stf[0][:], 0.0)
                    for t in range(T):
                        a_new, a_old = stf[(t + 1) % 2], stf[t % 2]
                        nc.vector._custom_dve(
                            LIF_CHARGE, out=a_new[:], in0=a_old[:],
                            in1=psf[:, t * BL : (t + 1) * BL],
                            s0=float(2.0**t), s1=0.0, imm2=float(2.0**t),
                        )
                        nc.vector._custom_dve(
                            SPIKE_GE, out=s_sb[:, m, t, :], in0=a_new[:],
                            s0=float(2.0 ** (t + 1)),
                        )
                pso = psE.tile([10, 128], F32, tag="pso")
                sv = s_sb.rearrange("p m t b -> p m (t b)")
                nc.tensor.matmul(pso[:], fc2w[:, 0, :], sv[:, 0, :],
                                 start=True, stop=False)
                nc.tensor.matmul(pso[:], fc2w[:, 1, :], sv[:, 1, :],
                                 start=False, stop=True)
                out_sb = pE.tile([10, 128], F32)
                nc.scalar.activation(out_sb[:], pso[:], AF.Identity,
                                     bias=fc2b[:, 0:1], scale=1.0)
                # transpose to [(t b), 10] so the output DMA has row-contiguous
                # descriptors instead of 1280 single-element ones
                psO = psE.tile([128, 128], F32, tag="psO")
                nc.tensor.transpose(psO[:, 0:10], out_sb[:], ident[0:10, 0:10])
                out2 = pE.tile([128, 10], F32)
                nc.scalar.copy(out2[:], psO[:, 0:10])
                nc.sync.dma_start(out_dr.ap().rearrange("t b o -> (t b) o"), out2[:])

    return nc


_CACHED = None


def _get_compiled():
    global _CACHED
    if _CACHED is None:
        nc = bacc.Bacc("TRN2", target_bir_lowering=False, debug=False,
                       num_devices=N_CORES)
        build(nc)
        nc.compile()
        _CACHED = nc
    return _CACHED


def kernel(**inputs) -> np.ndarray:
    nc = _get_compiled()
    np_in = {k: np.ascontiguousarray(np.asarray(v, dtype=np.float32))
             for k, v in inputs.items()}
    in_maps = []
    for i in range(N_CORES):
        m = dict(np_in)
        m["x_seq"] = np.ascontiguousarray(
            np_in["x_seq"][:, i * BL : (i + 1) * BL])
        in_maps.append(m)
    res = bass_utils.run_bass_kernel_spmd(nc, in_maps, core_ids=list(range(N_CORES)))
    return np.concatenate([res.results[i]["out"] for i in range(N_CORES)], axis=1)


if __name__ == "__main__":
    nc = _get_compiled()
    print("compiled OK")


# revision 25
# speedup vs baseline: 1.0804x; 1.0804x over previous
"""Trainium2 Bass kernel for nn_CifarBaselineSNN.

conv1(3->64,3x3,p1) -> BN -> LIF -> avgpool2
conv2(64->128,3x3,p1) -> BN -> LIF -> avgpool2
fc1(8192->256) -> LIF -> fc2(256->10)+b
T=8, B=128. Data-parallel over B across 8 NeuronCores (16 samples/core);
BN statistics are global over the batch -> 2 small AllReduces.

v2 layout: conv1 output is never stored to DRAM. Pass A1 computes conv1 only
to accumulate BN1 statistics; after the AllReduce, conv1 is recomputed fused
with the LIF1 scan (DVE reads PSUM directly) and conv2 runs in the same
per-timestep loop, overlapping TensorE and DVE. Weight loads use native-layout
DMAs plus on-chip TensorE transposes to avoid small-descriptor storms.
"""

import sys
import os

for _p in ("/opt/trn_rl_repo", "/root/.axon_site/_ro/trn_rl_repo"):
    if os.path.isdir(_p) and _p not in sys.path:
        sys.path.append(_p)

import numpy as np

import concourse.bass as bass
import concourse.mybir as mybir
import concourse.tile as tile
from concourse import bacc
from concourse import bass_utils
from concourse import masks
from concourse import dve_ops as _dops
from concourse.dve_uop import DveOpSpec
from concourse.dve_spec import (
    Spec, Src0, Src1, C0, C1, C2, Zero, AluOp, sq, select, lower, _has_src1,
)

F32 = mybir.dt.float32
BF16 = mybir.dt.bfloat16
AF = mybir.ActivationFunctionType
ALU = mybir.AluOpType

T = 8
B_FULL = 128
N_CORES = 8
BL = B_FULL // N_CORES  # 16 samples per core
EPS = 1e-5


# --------------------------------------------------------------------------
# Custom DVE ops (fused LIF pieces)
# --------------------------------------------------------------------------

def _register_op(name, spec, ref):
    for op in _dops.OPS:
        if op.name == name:
            return op
    spec = Spec(body=spec.body, accum=spec.accum, accum_init=spec.accum_init,
                reference=ref)
    shas = {}
    for ver in ("v3", "v4"):
        s = DveOpSpec(name=name, opcode=0, uops=lower(spec, ver=ver),
                      rd1_en=_has_src1(spec))
        shas[ver] = s.sha(ver)
    op = _dops.DveOp(name, spec, subdim=False, uops_sha=shas)
    _dops.OPS.append(op)
    _dops.CUSTOM_DVE_SPECS[name] = spec
    _dops._SUB_OPCODE_FOR_NAME[name] = max(_dops._SUB_OPCODE_FOR_NAME.values()) + 1
    return op


# A_t = (A_{t-1} if A_{t-1} < theta_{t-1} else 0) + y*scale + bias
LIF_CHARGE = _register_op(
    "LIF_CHARGE_SNN",
    Spec(body=select(Src0 >= C2, Zero, Src0) + Src1 * C0 + C1),
    lambda in0, in1, s0, s1, imm2: np.where(in0 >= imm2, 0.0, in0) + in1 * s0 + s1,
)

# spike counts over horizontal pairs: (a>=th) + (b>=th)   (values 0/1/2)
SPIKE_HPOOL = _register_op(
    "SPIKE_HPOOL_SNN",
    Spec(body=(Src0 >= C0) + (Src1 >= C0)),
    lambda in0, in1, s0, s1, imm2: (in0 >= s0).astype(np.float32)
    + (in1 >= s0).astype(np.float32),
)

# pooled = (hpA + hpB) * 0.25
VPOOL_SCALE = _register_op(
    "VPOOL_SCALE_SNN",
    Spec(body=(Src0 + Src1) * C1),
    lambda in0, in1, s0, s1, imm2: (in0 + in1) * s1,
)

# pre-scaled spike pair count: ((a>=th)+(b>=th)) * C1  -- lets the final
# 2x2 reduction be a plain ADD on the (otherwise idle) gpsimd engine
HPOOL_SC = _register_op(
    "HPOOL_SC_SNN",
    Spec(body=((Src0 >= C0) + (Src1 >= C0)) * C1),
    lambda in0, in1, s0, s1, imm2: ((in0 >= s0).astype(np.float32)
    + (in1 >= s0).astype(np.float32)) * s1,
)

# square + row-sum (for BN sum-of-squares on the vector engine)
SQ_ACC = _register_op(
    "SQ_ACC_SNN",
    Spec(body=sq(Src0), accum=AluOp.ADD),
    lambda in0, s0, s1, imm2: in0 * in0,
)

# plain spike: (a >= th)
SPIKE_GE = _register_op(
    "SPIKE_GE_SNN",
    Spec(body=(Src0 >= C0) + Zero),
    lambda in0, s0, s1, imm2: (in0 >= s0).astype(np.float32),
)


# --------------------------------------------------------------------------
# Kernel build
# --------------------------------------------------------------------------

def _stats_to_scale_bias(nc, pool, tot, g_dr, b_dr, n_count, nch, out_scale, out_bias):
    """tot: [nch,2] SBUF (sum, sumsq). Writes out_scale/out_bias [nch,8]:
    scale[:,t] = gamma*rstd*2^t ; bias[:,t] = (beta - mu*gamma*rstd)*2^t."""
    mu = pool.tile([nch, 1], F32)
    nc.vector.tensor_scalar_mul(mu[:], tot[:, 0:1], 1.0 / n_count)
    e2 = pool.tile([nch, 1], F32)
    nc.vector.tensor_scalar_mul(e2[:], tot[:, 1:2], 1.0 / n_count)
    var = pool.tile([nch, 1], F32)
    nc.vector.tensor_tensor(var[:], mu[:], mu[:], ALU.mult)
    nc.vector.tensor_tensor(var[:], e2[:], var[:], ALU.subtract)
    nc.vector.tensor_scalar_add(var[:], var[:], float(EPS))
    std = pool.tile([nch, 1], F32)
    nc.scalar.activation(std[:], var[:], AF.Sqrt, bias=0.0, scale=1.0)
    rstd = pool.tile([nch, 1], F32)
    nc.vector.reciprocal(rstd[:], std[:])
    gam = pool.tile([nch, 1], F32)
    nc.sync.dma_start(gam[:], g_dr.ap()[:, None])
    bet = pool.tile([nch, 1], F32)
    nc.sync.dma_start(bet[:], b_dr.ap()[:, None])
    gr = pool.tile([nch, 1], F32)
    nc.vector.tensor_tensor(gr[:], gam[:], rstd[:], ALU.mult)
    bb = pool.tile([nch, 1], F32)  # beta - mu*gr
    nc.vector.tensor_tensor(bb[:], mu[:], gr[:], ALU.mult)
    nc.vector.tensor_tensor(bb[:], bet[:], bb[:], ALU.subtract)
    for t in range(T):
        nc.vector.tensor_scalar_mul(out_scale[:nch, t : t + 1], gr[:], float(2.0**t))
        nc.vector.tensor_scalar_mul(out_bias[:nch, t : t + 1], bb[:], float(2.0**t))


def _allreduce(nc, dram_pool, sb_pool, src_ap, shape):
    """AllReduce-add src_ap ([P,F] SBUF) across all 8 cores; returns SBUF tile."""
    bin_ = dram_pool.tile(list(shape), F32)
    bout = dram_pool.tile(list(shape), F32)
    nc.gpsimd.dma_start(bin_[:], src_ap)
    nc.gpsimd.collective_compute(
        "AllReduce", ALU.add,
        replica_groups=[list(range(N_CORES))],
        ins=[bin_.opt()], outs=[bout.opt()],
    )
    res = sb_pool.tile(list(shape), F32)
    nc.gpsimd.dma_start(res[:], bout[:])
    return res


def build(nc):
    # ---- DRAM I/O -------------------------------------------------------
    x_seq = nc.dram_tensor("x_seq", [T, BL, 3, 32, 32], F32, kind="ExternalInput")
    w1_dr = nc.dram_tensor("conv1_w", [64, 3, 3, 3], F32, kind="ExternalInput")
    g1_dr = nc.dram_tensor("bn1_g", [64], F32, kind="ExternalInput")
    b1_dr = nc.dram_tensor("bn1_b", [64], F32, kind="ExternalInput")
    w2_dr = nc.dram_tensor("conv2_w", [128, 64, 3, 3], F32, kind="ExternalInput")
    g2_dr = nc.dram_tensor("bn2_g", [128], F32, kind="ExternalInput")
    b2_dr = nc.dram_tensor("bn2_b", [128], F32, kind="ExternalInput")
    fc1_dr = nc.dram_tensor("fc1_w", [256, 8192], F32, kind="ExternalInput")
    fc2_dr = nc.dram_tensor("fc2_w", [10, 256], F32, kind="ExternalInput")
    fc2b_dr = nc.dram_tensor("fc2_b", [10], F32, kind="ExternalInput")
    out_dr = nc.dram_tensor("out", [T, BL, 10], F32, kind="ExternalOutput")

    with tile.TileContext(nc) as tc:
        import contextlib
        with contextlib.ExitStack() as ctx:
            dram = ctx.enter_context(tc.tile_pool(name="dram", bufs=1, space="DRAM"))
            persist = ctx.enter_context(tc.tile_pool(name="persist", bufs=1))

            # DRAM scratch: conv2 pre-BN output, [t][ch][sample][pos]
            y2_dram = dram.tile([T, 128, BL, 256], BF16)

            # persistent small tensors
            scale1 = persist.tile([128, T], F32)
            bias1 = persist.tile([128, T], F32)
            scale2 = persist.tile([128, T], F32)
            bias2 = persist.tile([128, T], F32)
            s1buf = persist.tile([128, 32], F32)
            s2buf = persist.tile([128, 32], F32)
            s1buf2 = persist.tile([128, 32], F32)
            s2buf2 = persist.tile([128, 32], F32)
            ident = persist.tile([128, 128], F32)
            masks.make_identity(nc, ident[:])
            ident_bf = persist.tile([128, 128], BF16)
            masks.make_identity(nc, ident_bf[:])

            # conv1 weights: row r = g*3+ci (g=dy*3+dx); bf16 (conv1 runs in
            # pure bf16 -- layer-1 activations tolerate ~0.5% rounding, far
            # inside the LIF threshold margins)
            w1f = persist.tile([27, 64], F32)
            for dy in range(3):
                for dx in range(3):
                    r0 = dy * 9 + dx * 3
                    nc.sync.dma_start(
                        w1f[r0 : r0 + 3, :],
                        w1_dr.ap()[:, :, dy, dx].rearrange("c ci -> ci c"),
                    )
            w1b = persist.tile([27, 64], BF16)
            nc.vector.tensor_copy(w1b[:], w1f[:])

            # conv2 weights: native load [c,(ci g)] then 9 TensorE transposes
            # of the stride-9 column slices -> psum [64ci, 128c]; hi/lo split.
            # w2cat rows 0-63 = W_hi(ci), 64-127 = W_lo(ci).
            w2cat = ctx.enter_context(tc.tile_pool(name="w2pool", bufs=1)).tile(
                [128, 9, 128], BF16)
            with tc.tile_pool(name="w2stage", bufs=1) as w2st, \
                 tc.tile_pool(name="w2ps", bufs=2, space="PSUM") as w2ps:
                w2nat = w2st.tile([128, 576], F32)
                nc.sync.dma_start(w2nat[:], w2_dr.ap().rearrange("c ci dy dx -> c (ci dy dx)"))
                w2f2 = w2st.tile([64, 9, 128], F32)
                for g in range(9):
                    psT = w2ps.tile([128, 512], F32, tag="w2t", name="w2t")
                    nc.tensor.transpose(psT[0:64, 0:128],
                                        w2nat[:, g : 576 : 9], ident[:, 0:128])
                    nc.scalar.copy(w2f2[:, g, :], psT[0:64, 0:128])
                w2lo = w2st.tile([64, 9, 128], BF16)
                nc.vector.tensor_copy(w2cat[0:64], w2f2[:])
                nc.vector.tensor_tensor(w2lo[:], w2f2[:], w2cat[0:64], ALU.subtract)
                nc.sync.dma_start(w2cat[64:128], w2lo[:])

            # =============== x staging: padded bf16 hi/lo planes in DRAM ====
            GUARD = 64
            TPLANE = BL * 1156  # one timestep's stream of padded 34x34 frames
            TSTR = GUARD + TPLANE + GUARD
            # per-t DRAM tiles: imc(t) only depends on its own 3 staging DMAs
            xbase = [dram.tile([3, TSTR], BF16, name=f"xb{t}") for t in range(T)]
            st_engs = [nc.sync, nc.scalar, nc.gpsimd]
            with tc.tile_pool(name="xstage", bufs=1) as xst:
                xpadF = xst.tile([128, 3, 34, 34], F32)
                nc.vector.memset(xpadF[:], 0.0)
                xraw = xst.tile([128, 3, 32, 32], F32)
                nc.sync.dma_start(
                    xraw.rearrange("p c h w -> p (c h w)"),
                    x_seq.ap().rearrange("t b c h w -> (t b) (c h w)"),
                )
                nc.scalar.copy(xpadF[:, :, 1:33, 1:33], xraw[:])
                xpad_flat = xpadF.rearrange("p c h w -> p (c h w)")
                x_hiF = xst.tile([128, 3, 34, 34], BF16)
                xhi_flat = x_hiF.rearrange("p c h w -> p (c h w)")
                nc.vector.tensor_copy(xhi_flat[:], xpad_flat[:])
                for t in range(T):
                    for ci in range(3):
                        st_engs[(3 * t + ci) % 3].dma_start(
                            xbase[t][ci, GUARD : GUARD + TPLANE]
                                .rearrange("(b f) -> b f", f=1156),
                            x_hiF[t * BL : (t + 1) * BL, ci]
                                .rearrange("p h w -> p (h w)"),
                        )

            # im2col row r = g*3+ci reads plane ci at shift (dy-1)*34+(dx-1);
            # one 2-d AP [ci:TSTR x cols:1] per 3-row block. Per-t tiles.
            TCOLS = TPLANE

            def imc_src(t, dy, dx):
                a = xbase[t][:]
                off = (a.offset + GUARD + (dy - 1) * 34 + (dx - 1))
                return bass.AP(a.tensor, off, [[TSTR, 3], [1, TCOLS]])

            imc_ctx = contextlib.ExitStack()
            imc_pool = imc_ctx.enter_context(tc.tile_pool(name="imc", bufs=2))

            imc_engs = [nc.sync, nc.scalar, nc.gpsimd]

            def load_imc(t):
                imc = imc_pool.tile([27, TCOLS], BF16, tag="imc", name="imc")
                for g in range(9):
                    dy, dx = g // 3, g % 3
                    r0 = 3 * g
                    imc_engs[g % 3].dma_start(imc[r0 : r0 + 3, :],
                                              imc_src(t, dy, dx))
                return imc.rearrange("p (b h w) -> p b h w", h=34, w=34)

            def conv1_mm(ps, imc_v, p):
                # ps [128,1024] = [2 samples x 64ch, 32x32]; 4 matmuls
                for bhalf in range(2):
                    fr = 2 * p + bhalf
                    for hh in range(2):
                        h0 = hh * 16
                        rhs = imc_v[:, fr, h0 + 1 : h0 + 17, 1:33]
                        nc.tensor.matmul(
                            ps[64 * bhalf : 64 * bhalf + 64,
                               512 * hh : 512 * hh + 512],
                            w1b[:, :], rhs[0:27],
                            start=True, stop=True,
                            tile_position=(0, 64 * bhalf),
                        )

            # =============== PASS A1: conv1 -> BN1 stats only ===============
            # 4 samples per [128,2048] psum tile halves the per-drain
            # semaphore chains (stats ops amortize handoff latency better)
            with tc.tile_pool(name="psumA1", bufs=2, space="PSUM") as psA1, \
                 tc.tile_pool(name="yscr", bufs=2) as yscr_pool, \
                 tc.tile_pool(name="sqscr", bufs=2) as sq_pool:
                for t in range(T):
                    imc_v = load_imc(t)
                    for pp in range(4):
                        idx = t * 4 + pp
                        ps = psA1.tile([128, 2048], F32, tag="ps", name="psA1")
                        for jj in range(4):
                            fr = 4 * pp + jj
                            for hh in range(2):
                                h0 = hh * 16
                                rhs = imc_v[:, fr, h0 + 1 : h0 + 17, 1:33]
                                nc.tensor.matmul(
                                    ps[64 * (jj % 2) : 64 * (jj % 2) + 64,
                                       (jj // 2) * 1024 + 512 * hh :
                                       (jj // 2) * 1024 + 512 * hh + 512],
                                    w1b[:, :], rhs[0:27],
                                    start=True, stop=True,
                                    tile_position=(0, 64 * (jj % 2)),
                                )
                        y_scr = yscr_pool.tile([128, 2048], F32, name="yscr")
                        nc.scalar.activation(y_scr[:], ps[:], AF.Identity,
                                             bias=0.0, scale=1.0,
                                             accum_out=s1buf[:, idx : idx + 1])
                        sq_t = sq_pool.tile([128, 2048], F32, name="sqscr")
                        nc.vector._custom_dve(
                            SQ_ACC, out=sq_t[:], in0=ps[:],
                            accum_out=s2buf[:, idx : idx + 1])

            # =============== BN1 stats + allreduce ===============
            sums1 = persist.tile([128, 2], F32)
            nc.vector.tensor_reduce(sums1[:, 0:1], s1buf[:], mybir.AxisListType.X, ALU.add)
            nc.vector.tensor_reduce(sums1[:, 1:2], s2buf[:], mybir.AxisListType.X, ALU.add)
            g1 = _allreduce(nc, dram, persist, sums1[:], (128, 2))
            par1 = persist.tile([64, 2], F32)
            nc.sync.dma_start(par1[:], g1[64:128, :])
            tot1 = persist.tile([64, 2], F32)
            nc.vector.tensor_tensor(tot1[:], g1[0:64, :], par1[:], ALU.add)
            _stats_to_scale_bias(nc, persist, tot1, g1_dr, b1_dr,
                                 float(T * B_FULL * 32 * 32), 64, scale1, bias1)
            nc.sync.dma_start(scale1[64:128, :], scale1[0:64, :])
            nc.sync.dma_start(bias1[64:128, :], bias1[0:64, :])

            # =============== FUSED A2+B+C: conv1 -> LIF1 -> pool -> conv2 ===
            with tc.tile_pool(name="stfused", bufs=1) as stp, \
                 tc.tile_pool(name="p1pool", bufs=1) as p1p, \
                 tc.tile_pool(name="psumA2", bufs=2, space="PSUM") as psA2, \
                 tc.tile_pool(name="psumC", bufs=2, space="PSUM") as psC, \
                 tc.tile_pool(name="hp1", bufs=1) as hp_pool, \
                 tc.tile_pool(name="dup", bufs=2) as dup_pool, \
                 tc.tile_pool(name="ysb2", bufs=2) as ysb2_pool, \
                 tc.tile_pool(name="sq2", bufs=2) as sq2_pool:
                st = [stp.tile([128, 8, 1024], F32, name=f"st{i}") for i in range(2)]
                nc.vector.memset(st[0].rearrange("p a b -> p (a b)"), 0.0)
                pooled1 = [p1p.tile([128, 8, 18, 18], BF16, name=f"p1_{i}")
                           for i in range(2)]
                nc.vector.memset(pooled1[0].rearrange("p a h w -> p (a h w)"), 0.0)
                nc.vector.memset(pooled1[1].rearrange("p a h w -> p (a h w)"), 0.0)

                for t in range(T):
                    imc_v = load_imc(t)
                    a_old, a_new = st[t % 2], st[(t + 1) % 2]
                    p1t = pooled1[t % 2]
                    for p in range(8):
                        ps = psA2.tile([128, 1024], F32, tag="ps2", name="psA2")
                        conv1_mm(ps, imc_v, p)
                        nc.vector._custom_dve(
                            LIF_CHARGE, out=a_new[:, p, :], in0=a_old[:, p, :],
                            in1=ps[:],
                            s0=scale1[:, t : t + 1], s1=bias1[:, t : t + 1],
                            imm2=float(2.0**t),
                        )
                        av = a_new[:, p, :].rearrange("p (h w) -> p h w", h=32)
                        hp = hp_pool.tile([128, 32, 16], F32, tag="hp", name="hp")
                        nc.vector._custom_dve(
                            HPOOL_SC, out=hp[:],
                            in0=av[:, :, 0:32:2], in1=av[:, :, 1:32:2],
                            s0=float(2.0 ** (t + 1)), s1=0.25,
                        )
                        nc.vector._custom_dve(
                            VPOOL_SCALE,
                            out=p1t[:, p, 1:17, 1:17],
                            in0=hp[:, 0:32:2, :], in1=hp[:, 1:32:2, :],
                            s1=1.0,
                        )
                    # conv2 for this timestep: duplicate (ci | ci) partitions,
                    # samples blocked evens (slots 0-7) then odds (slots 8-15)
                    dup = dup_pool.tile([128, 16, 324], BF16, tag="dup", name="dup")
                    p1flat = [p1t[0:64].rearrange("p a h w -> p a (h w)"),
                              p1t[64:128].rearrange("p a h w -> p a (h w)")]
                    nc.sync.dma_start(dup[0:64, 0:8, :], p1flat[0])
                    nc.scalar.dma_start(dup[64:128, 0:8, :], p1flat[0])
                    nc.gpsimd.dma_start(dup[0:64, 8:16, :], p1flat[1])
                    nc.sync.dma_start(dup[64:128, 8:16, :], p1flat[1])
                    dup_v = dup.rearrange("p s (h w) -> p s h w", h=18)
                    y_sb = ysb2_pool.tile([128, BL, 256], BF16, tag="ysb",
                                          name="ysb2")
                    for sp2 in range(4):
                        cidx = t * 4 + sp2
                        ps2 = psC.tile([128, 1024], F32, tag="psc", name="psC")
                        for half2 in range(2):
                            j0 = 2 * sp2 + half2
                            for g in range(9):
                                dy, dx = g // 3, g % 3
                                nc.tensor.matmul(
                                    ps2[:, 512 * half2 : 512 * half2 + 512],
                                    w2cat[:, g, :],
                                    dup_v[:, j0 : j0 + 9 : 8,
                                          dy : dy + 16, dx : dx + 16],
                                    start=(g == 0), stop=(g == 8),
                                )
                        nc.scalar.activation(
                            y_sb[:, 4 * sp2 : 4 * sp2 + 4, :], ps2[:],
                            AF.Identity, bias=0.0, scale=1.0,
                            accum_out=s1buf2[:, cidx : cidx + 1])
                        sq_t = sq2_pool.tile([128, 1024], F32, name="sq2")
                        nc.vector._custom_dve(
                            SQ_ACC, out=sq_t[:], in0=ps2[:],
                            accum_out=s2buf2[:, cidx : cidx + 1])
                    st_engs[t % 3].dma_start(y2_dram[t], y_sb[:])

            imc_ctx.close()  # free the 74KB/partition im2col buffers

            # =============== BN2 stats + allreduce ===============
            sums2 = persist.tile([128, 2], F32)
            nc.vector.tensor_reduce(sums2[:, 0:1], s1buf2[:], mybir.AxisListType.X, ALU.add)
            nc.vector.tensor_reduce(sums2[:, 1:2], s2buf2[:], mybir.AxisListType.X, ALU.add)
            g2 = _allreduce(nc, dram, persist, sums2[:], (128, 2))
            _stats_to_scale_bias(nc, persist, g2, g2_dr, b2_dr,
                                 float(T * B_FULL * 16 * 16), 128, scale2, bias2)

            # fc weights: native loads + TensorE transposes -> fp32 lhsT tiles
            # fc1wT[:, hw, m, :] = fc1_w[m*128:(m+1)*128, c*64+hw]^T  ([c, o])
            fcpool = ctx.enter_context(tc.tile_pool(name="fcpool", bufs=1))
            fc1wT = fcpool.tile([128, 64, 2, 128], F32)
            fc2w = fcpool.tile([128, 2, 10], F32)
            fc2b = fcpool.tile([10, 1], F32)
            nc.sync.dma_start(fc2b[:], fc2b_dr.ap()[:, None])
            with tc.tile_pool(name="fcstage", bufs=1) as fst, \
                 tc.tile_pool(name="psumT", bufs=4, space="PSUM") as psT_pool:
                fc2nat = fst.tile([10, 256], F32)
                nc.sync.dma_start(fc2nat[:], fc2_dr.ap())
                for m in range(2):
                    psT = psT_pool.tile([128, 512], F32, tag="fct", name="fct")
                    nc.tensor.transpose(psT[:, 0:10],
                                        fc2nat[:, m * 128 : (m + 1) * 128],
                                        ident[0:10, 0:10])
                    nc.scalar.copy(fc2w[:, m, :], psT[:, 0:10])
                for m in range(2):
                    fc1nat = fst.tile([128, 8192], F32, tag="fcn", name="fcn")
                    # ordering hack: prime one element from the last conv2
                    # stats column so this 8MB load is not hoisted into the
                    # DMA-bound conv windows
                    nc.scalar.dma_start(fc1nat[0:1, 0:1], s2buf2[0:1, 31:32])
                    nc.scalar.dma_start(fc1nat[:], fc1_dr.ap()[m * 128 : (m + 1) * 128, :])
                    for hw in range(64):
                        psT = psT_pool.tile([128, 512], F32, tag="fct", name="fct")
                        nc.tensor.transpose(psT[:, 0:128],
                                            fc1nat[:, hw : 8192 : 64],
                                            ident[:, 0:128])
                        nc.scalar.copy(fc1wT[:, hw, m, :], psT[:, 0:128])

                # ======== STAGE D: LIF2 + pool (all 16 samples per op) ======
                pooled2 = fcpool.tile([128, T, BL, 8, 8], F32)
                with tc.tile_pool(name="stageD", bufs=2) as pD, \
                     tc.tile_pool(name="stD", bufs=1) as stD, \
                     tc.tile_pool(name="hp2", bufs=1) as hp2_pool:
                    st2 = [stD.tile([128, 4096], F32, name=f"st2_{i}") for i in range(2)]
                    nc.vector.memset(st2[0][:], 0.0)
                    for t in range(T):
                        yc = pD.tile([128, 4096], BF16, tag="yc", name="ycD")
                        st_engs[t % 3].dma_start(
                            yc.rearrange("p (b c) -> p b c", b=BL),
                            y2_dram[t],
                        )
                        a_old, a_new = st2[t % 2], st2[(t + 1) % 2]
                        nc.vector._custom_dve(
                            LIF_CHARGE, out=a_new[:], in0=a_old[:], in1=yc[:],
                            s0=scale2[:, t : t + 1], s1=bias2[:, t : t + 1],
                            imm2=float(2.0**t),
                        )
                        # state flat = (b,h,w); (b h) collapses to one stride-16 dim
                        av = a_new.rearrange("p (bh w) -> p bh w", w=16)
                        hp2 = hp2_pool.tile([128, BL * 16, 8], F32, tag="hp2", name="hp2")
                        nc.vector._custom_dve(
                            HPOOL_SC, out=hp2[:],
                            in0=av[:, :, 0:16:2], in1=av[:, :, 1:16:2],
                            s0=float(2.0 ** (t + 1)), s1=0.25,
                        )
                        # hp2 flat = (b,h,w2): rows h even/odd pair up as
                        # offset j*16+{0..7} and j*16+{8..15} with j=(b,h')
                        hv = hp2.rearrange("p j w -> p (j w)").rearrange(
                            "p (j k) -> p j k", k=16)
                        p2o = pooled2.rearrange("p t b h w -> p (t b h) w")
                        nc.vector._custom_dve(
                            VPOOL_SCALE,
                            out=p2o[:, t * BL * 8 : (t + 1) * BL * 8, :],
                            in0=hv[:, :, 0:8], in1=hv[:, :, 8:16],
                            s1=1.0,
                        )

            # =============== STAGE E: fc1 (fp32) + LIF + fc2 ===============
            p2v = pooled2.rearrange("p t b h w -> p (t b) (h w)")
            with tc.tile_pool(name="stageE", bufs=1) as pE, \
                 tc.tile_pool(name="psumE", bufs=2, space="PSUM") as psE:
                s_sb = pE.tile([128, 2, T, BL], F32)
                for m in range(2):
                    psf = psE.tile([128, 128], F32, tag="psf")
                    for hw in range(64):
                        nc.tensor.matmul(
                            psf[:], fc1wT[:, hw, m, :], p2v[:, :, hw],
                            start=(hw == 0), stop=(hw == 63),
                        )
                    stf = [pE.tile([128, BL], F32, tag=f"stf{i}", name=f"stf{i}")
                           for i in range(2)]
                    nc.vector.memset(stf[0][:], 0.0)
                    for t in range(T):
                        a_new, a_old = stf[(t + 1) % 2], stf[t % 2]
                        nc.vector._custom_dve(
                            LIF_CHARGE, out=a_new[:], in0=a_old[:],
                            in1=psf[:, t * BL : (t + 1) * BL],
                            s0=float(2.0**t), s1=0.0, imm2=float(2.0**t),
                        )
                        nc.vector._custom_dve(
                            SPIKE_GE, out=s_sb[:, m, t, :], in0=a_new[:],
                            s0=float(2.0 ** (t + 1)),
                        )
                pso = psE.tile([10, 128], F32, tag="pso")
                sv = s_sb.rearrange("p m t b -> p m (t b)")
                nc.tensor.matmul(pso[:], fc2w[:, 0, :], sv[:, 0, :],
                                 start=True, stop=False)
                nc.tensor.matmul(pso[:], fc2w[:, 1, :], sv[:, 1, :],
                                 start=False, stop=True)
                out_sb = pE.tile([10, 128], F32)
                nc.scalar.activation(out_sb[:], pso[:], AF.Identity,
                                     bias=fc2b[:, 0:1], scale=1.0)
                # transpose to [(t b), 10] so the output DMA has row-contiguous
                # descriptors instead of 1280 single-element ones
                psO = psE.tile([128, 128], F32, tag="psO")
                nc.tensor.transpose(psO[:, 0:10], out_sb[:], ident[0:10, 0:10])
                out2 = pE.tile([128, 10], F32)
                nc.scalar.copy(out2[:], psO[:, 0:10])
                nc.sync.dma_start(out_dr.ap().rearrange("t b o -> (t b) o"), out2[:])

    return nc


_CACHED = None


def _get_compiled():
    global _CACHED
    if _CACHED is None:
        nc = bacc.Bacc("TRN2", target_bir_lowering=False, debug=False,
                       num_devices=N_CORES)
        build(nc)
        nc.compile()
        _CACHED = nc
    return _CACHED


def kernel(**inputs) -> np.ndarray:
    nc = _get_compiled()
    np_in = {k: np.ascontiguousarray(np.asarray(v, dtype=np.float32))
             for k, v in inputs.items()}
    in_maps = []
    for i in range(N_CORES):
        m = dict(np_in)
        m["x_seq"] = np.ascontiguousarray(
            np_in["x_seq"][:, i * BL : (i + 1) * BL])
        in_maps.append(m)
    res = bass_utils.run_bass_kernel_spmd(nc, in_maps, core_ids=list(range(N_CORES)))
    return np.concatenate([res.results[i]["out"] for i in range(N_CORES)], axis=1)


if __name__ == "__main__":
    nc = _get_compiled()
    print("compiled OK")
